# revision 3
# baseline (speedup 1.0000x reference)
"""Paged multi-head attention decode step on 8 trn2 NeuronCores.

Sharding: tensor-parallel over heads. Core c owns heads [4c, 4c+4):
  - rows  [512c, 512(c+1)) of Wq/Wk/Wv, cols [512c, 512(c+1)) of Wo
  - head-slice of the (gathered, per-sequence) KV cache
Each core computes q/k/v for its heads for all 8 sequences, injects the new
token's k/v into its KV tiles, runs softmax(q K^T / sqrt(d)) V over the valid
context, then a partial output projection out_c = ctx_c @ Wo_c.  The full
output is the sum over cores (done on host).

All HBM-streamed operands (x, Wq/Wk/Wv/Wo slices, gathered K/V) are cast to
bf16 on the host — halves HBM traffic (the binding resource; ~31 MB/core) and
enables PE fast-weight-load.  PSUM accumulation stays fp32; softmax
denominators and the context normalization stay fp32.

PE cost shape (ldweights ~ columns/1.2 GHz, so the stationary operand must be
the FEW-column one wherever possible):
  - projections: x^T chunk [128k, 8b] stationary (8-col LDW), W^T chunk
    [128k, 512j] moving -> q/k/v land row-major [8, 512] in psum; cheap PE
    transposes produce the column forms attention needs.
  - scores: K chunk [128d, 128t] stationary (128-col LDW but bf16 FWL),
    q column moving; out [128 tok, 4 h] -> exp -> bf16 attn tile.
  - PV: attn [128t, 4h] stationary (4-col LDW), V [128t, 512d] moving.
  - Wo: ctxT [128d, 8b] stationary (8-col LDW), Wo^T [128d, 512j] moving.

Sequence lengths (positions) are host-known at trace time, so all loop trip
counts are static and the kernel only reads the valid (128-padded) context.
"""

import math

import numpy as np
import ml_dtypes

import concourse.bass as bass
import concourse.mybir as mybir
import concourse.tile as tile
from concourse import bacc
from concourse.bass_utils import run_bass_kernel_spmd
from concourse.masks import make_identity

BLOCK_SIZE = 16
NUM_HEADS = 32
HEAD_DIM = 128
D_MODEL = NUM_HEADS * HEAD_DIM
B = 8
N_CORES = 8
H_LOC = NUM_HEADS // N_CORES          # 4 heads per core
KSLICE = H_LOC * HEAD_DIM             # 512 contraction slice per core
NPAIR = H_LOC * B                     # 32 (seq, head) pairs per core
SCALE = 1.0 / math.sqrt(HEAD_DIM)

_F32 = mybir.dt.float32
_BF16 = mybir.dt.bfloat16


def _cfg_from_positions(pos):
    pos = [int(p) for p in pos]
    tpad = [((p + 1) + 127) // 128 * 128 for p in pos]
    nt = [t // 128 for t in tpad]
    kofs = np.concatenate([[0], np.cumsum([4 * t for t in tpad])]).tolist()
    vofs = np.concatenate([[0], np.cumsum([512 * n for n in nt])]).tolist()
    return {
        "pos": pos, "tpad": tpad, "nt": nt,
        "kofs": kofs, "vofs": vofs,
        "sumk": int(kofs[-1]), "sumv": int(vofs[-1]),
        "maxnt": max(nt),
    }


def _bcast_pairs(nc, psp, const, col, ones, ident, name):
    """[NPAIR,1] column -> [128, NPAIR] sbuf tile with the value of pair j
    replicated down all 128 partitions of column j (via PE transpose + ones
    outer-product)."""
    t1 = psp.tile([1, NPAIR], _F32, tag="ps", name=f"{name}_t1")
    nc.tensor.transpose(t1[:], col[:], ident[0:NPAIR, 0:NPAIR])
    row = const.tile([1, NPAIR], _F32, tag=f"{name}_row", name=f"{name}_row")
    nc.vector.tensor_copy(out=row[:], in_=t1[:])
    t2 = psp.tile([128, NPAIR], _F32, tag="ps", name=f"{name}_t2")
    nc.tensor.matmul(t2[:], lhsT=ones[:], rhs=row[:], start=True, stop=True)
    bc = const.tile([128, NPAIR], _F32, tag=f"{name}_bc", name=f"{name}_bc")
    nc.vector.tensor_copy(out=bc[:], in_=t2[:])
    return bc


def _build(cfg, nrep=1):
    pos, tpad, nt = cfg["pos"], cfg["tpad"], cfg["nt"]
    kofs, vofs = cfg["kofs"], cfg["vofs"]
    maxnt = cfg["maxnt"]
    tmax = 128 * maxnt

    nc = bacc.Bacc("TRN2", target_bir_lowering=False, debug=False)

    xt_d = nc.dram_tensor("xt", [128, 32 * B], _BF16, kind="ExternalInput")
    wqkv_d = nc.dram_tensor("wqkv", [128, 3 * 32 * 512], _BF16, kind="ExternalInput")
    wo_d = nc.dram_tensor("wo_t", [128, 32 * 512], _BF16, kind="ExternalInput")
    kt_d = nc.dram_tensor("kt", [128, cfg["sumk"]], _BF16, kind="ExternalInput")
    vg_d = nc.dram_tensor("vg", [128, cfg["sumv"]], _BF16, kind="ExternalInput")
    out_d = nc.dram_tensor("out_part", [B, D_MODEL], _F32, kind="ExternalOutput")

    with tile.TileContext(nc) as tc:
        with (
            tc.tile_pool(name="const", bufs=1) as const,
            tc.tile_pool(name="wstream", bufs=3) as wpool,
            tc.tile_pool(name="wostream", bufs=4) as wopool,
            tc.tile_pool(name="kstream", bufs=3) as kpool,
            tc.tile_pool(name="vstream", bufs=3) as vpool,
            tc.tile_pool(name="ps", bufs=8, space="PSUM") as psp,
        ):
            ident = const.tile([128, 128], _F32, tag="ident")
            make_identity(nc, ident[:])
            ones = const.tile([1, 128], _F32, tag="ones")
            nc.vector.memset(ones[:], 1.0)

            for rep in range(nrep):
                xt_sb = const.tile([128, 32, B], _BF16, tag="xt")
                nc.sync.dma_start(
                    out=xt_sb[:], in_=xt_d.ap().rearrange("p (c b) -> p c b", b=B)
                )

                # ---- Q,K,V projections, classic form: x^T chunk stationary
                # (8-col LDW), W^T chunk moving (512 cols).  q/k/v land
                # row-major [8 b, 512 (h d)] in psum, accumulated over the 32
                # k-chunks streaming in 4 x 1MB DMAs per projection.
                rows = []  # sbuf row-major copies: q_sb, k_sb (f32), v_bf
                for p_i, pname in enumerate(("q", "k", "v")):
                    ps = psp.tile([B, KSLICE], _F32, tag="ps", name=f"ps_{pname}")
                    for g in range(4):
                        wt = wpool.tile([128, 4096], _BF16, tag="w",
                                        name=f"wt_{pname}{g}")
                        nc.sync.dma_start(
                            out=wt[:],
                            in_=wqkv_d.ap()[:, (4 * p_i + g) * 4096:
                                            (4 * p_i + g + 1) * 4096],
                        )
                        for j in range(8):
                            i = 8 * g + j
                            nc.tensor.matmul(
                                ps[:], lhsT=xt_sb[:, i, :],
                                rhs=wt[:, 512 * j: 512 * (j + 1)],
                                start=(i == 0), stop=(i == 31),
                            )
                    if pname == "v":
                        v_bf = const.tile([B, KSLICE], _BF16, tag="v_bf")
                        nc.scalar.copy(out=v_bf[:], in_=ps[:])
                        rows.append(v_bf)
                    else:
                        row = const.tile([B, KSLICE], _F32, tag=f"{pname}_sb")
                        nc.vector.tensor_copy(out=row[:], in_=ps[:])
                        rows.append(row)
                q_sb, k_sb, v_bf = rows

                # ---- transpose q,k row-form -> column form [128 d, 8h+b] bf16
                qT = const.tile([128, NPAIR], _BF16, tag="qT")
                kT = const.tile([128, NPAIR], _BF16, tag="kT")
                for src, dst in ((q_sb, qT), (k_sb, kT)):
                    for h in range(H_LOC):
                        tp = psp.tile([128, B], _F32, tag="ps", name=f"tp{h}")
                        nc.tensor.transpose(
                            tp[:], src[0:B, 128 * h: 128 * (h + 1)], ident[0:B, 0:B]
                        )
                        nc.vector.tensor_copy(
                            out=dst[:, 8 * h: 8 * h + B], in_=tp[:]
                        )

                # ---- attention, streamed per sequence (one-pass softmax;
                # scores are O(1) so exp needs no max-shift).
                ctxT = const.tile([128, NPAIR], _F32, tag="ctxT")  # col = 8h+b
                psums = const.tile([128, NPAIR], _F32, tag="psums")
                for b in range(B):
                    kt_t = kpool.tile([128, H_LOC, tmax], _BF16, tag="kt",
                                      name=f"kt{b}")
                    nc.sync.dma_start(
                        out=kt_t[:, :, 0:tpad[b]],
                        in_=kt_d.ap()[:, kofs[b]: kofs[b] + 4 * tpad[b]]
                        .rearrange("p (h t) -> p h t", h=H_LOC),
                    )
                    vt = vpool.tile([128, maxnt, 512], _BF16, tag="vt",
                                    name=f"vt{b}")
                    nc.sync.dma_start(
                        out=vt[:, 0:nt[b], :],
                        in_=vg_d.ap()[:, vofs[b]: vofs[b] + 512 * nt[b]]
                        .rearrange("p (c f) -> p c f", f=512),
                    )
                    # inject the new token's k (column pos) and v (row pos)
                    nc.vector.tensor_copy(
                        out=kt_t[:, :, pos[b]],
                        in_=kT[:].rearrange("p (h b) -> p b h", b=B)[:, b, :],
                    )
                    nc.sync.dma_start(
                        out=vt[pos[b] % 128: pos[b] % 128 + 1, pos[b] // 128, :],
                        in_=v_bf[b: b + 1, :],
                    )

                    attn_b = kpool.tile([128, nt[b], H_LOC], _BF16, tag="attn",
                                        name=f"attn{b}", bufs=2)
                    ct = psp.tile([H_LOC, KSLICE], _F32, tag="ps", name=f"ct{b}")
                    for tt in range(nt[b]):
                        sc = psp.tile([128, H_LOC], _F32, tag="ps",
                                      name=f"sc{b}_{tt}")
                        for h in range(H_LOC):
                            nc.tensor.matmul(
                                sc[:, h: h + 1],
                                lhsT=kt_t[:, h, 128 * tt: 128 * (tt + 1)],
                                rhs=qT[:, 8 * h + b: 8 * h + b + 1],
                                start=(h == 0), stop=(h == H_LOC - 1),
                            )
                        nc.scalar.activation(
                            out=attn_b[:, tt, :], in_=sc[:],
                            func=mybir.ActivationFunctionType.Exp,
                        )
                        if tt == nt[b] - 1 and pos[b] % 128 != 127:
                            # zero invalid rows p > pos%128: keep where r-p >= 0
                            nc.gpsimd.affine_select(
                                out=attn_b[:, tt, :], in_=attn_b[:, tt, :],
                                compare_op=mybir.AluOpType.is_ge,
                                fill=0.0,
                                base=pos[b] % 128,
                                pattern=[[0, H_LOC]],
                                channel_multiplier=-1,
                            )
                        nc.tensor.matmul(
                            ct[:],
                            lhsT=attn_b[:, tt, :],
                            rhs=vt[:, tt, :],
                            start=(tt == 0), stop=(tt == nt[b] - 1),
                        )
                    # per-seq partial softmax denominators (sum over tiles)
                    nc.vector.reduce_sum(
                        out=psums[:, 4 * b: 4 * b + 4],
                        in_=attn_b[:].rearrange("p c j -> p j c"),
                        axis=mybir.AxisListType.X,
                    )
                    ct_sb = const.tile([H_LOC, KSLICE], _F32, tag="ct_sb",
                                       name=f"ct_sb{b}", bufs=2)
                    nc.vector.tensor_copy(out=ct_sb[:], in_=ct[:])
                    for h in range(H_LOC):
                        ctt = psp.tile([128, H_LOC], _F32, tag="ps",
                                       name=f"ctt{b}_{h}")
                        nc.tensor.transpose(
                            ctt[:], ct_sb[0:H_LOC, 128 * h: 128 * (h + 1)],
                            ident[0:H_LOC, 0:H_LOC],
                        )
                        nc.vector.tensor_copy(
                            out=ctxT[:, 8 * h + b: 8 * h + b + 1],
                            in_=ctt[:, h: h + 1],
                        )

                # ---- 1/sum per pair, broadcast down partitions, normalize
                psums_t = psp.tile([NPAIR, 128], _F32, tag="ps", name="psums_t")
                nc.tensor.transpose(psums_t[:], psums[:], ident[:])
                denom = const.tile([NPAIR, 1], _F32, tag="denom")
                nc.vector.reduce_sum(out=denom[:], in_=psums_t[:],
                                     axis=mybir.AxisListType.X)
                recip = const.tile([NPAIR, 1], _F32, tag="recip")
                nc.vector.reciprocal(recip[:], denom[:])
                rc_bc = _bcast_pairs(nc, psp, const, recip, ones, ident, "rc")
                # recip ordered by pair=4b+h; ctxT cols are 8h+b -> permute
                nc.vector.tensor_mul(
                    ctxT[:].rearrange("p (h b) -> p h b", b=B),
                    ctxT[:].rearrange("p (h b) -> p h b", b=B),
                    rc_bc[:].rearrange("p (b h) -> p h b", h=H_LOC),
                )
                ctx_bf = const.tile([128, NPAIR], _BF16, tag="ctx_bf")
                nc.vector.tensor_copy(out=ctx_bf[:], in_=ctxT[:])

                # ---- output projection partial:
                # out[b, 512n+j] = sum_h ctxT[:, 8h+b]^T wo[:, (n,h,j)]
                outsb = const.tile([B, D_MODEL], _F32, tag="outsb")
                for g in range(4):
                    wot = wopool.tile([128, 4096], _BF16, tag="wo", name=f"wo{g}")
                    nc.sync.dma_start(
                        out=wot[:], in_=wo_d.ap()[:, g * 4096: (g + 1) * 4096]
                    )
                    for m in range(2):
                        n = 2 * g + m
                        op = psp.tile([B, 512], _F32, tag="ps", name=f"op{n}")
                        for h in range(H_LOC):
                            nc.tensor.matmul(
                                op[:],
                                lhsT=ctx_bf[:, 8 * h: 8 * h + B],
                                rhs=wot[:, 2048 * m + 512 * h:
                                        2048 * m + 512 * (h + 1)],
                                start=(h == 0), stop=(h == H_LOC - 1),
                            )
                        nc.scalar.copy(
                            out=outsb[:, 512 * n: 512 * (n + 1)], in_=op[:]
                        )
                nc.sync.dma_start(out=out_d.ap(), in_=outsb[:])

    nc.compile()
    return nc


_PROGRAM_CACHE = {}


def _get_program(cfg, nrep=1):
    key = (tuple(cfg["pos"]), nrep)
    if key not in _PROGRAM_CACHE:
        _PROGRAM_CACHE[key] = _build(cfg, nrep=nrep)
    return _PROGRAM_CACHE[key]


def _bf(a):
    return np.asarray(a, dtype=ml_dtypes.bfloat16)


def make_core_inputs(cfg, c, x, Wq, Wk, Wv, Wo, key_cache, value_cache,
                     block_tables):
    """Host-side shard prep for core c."""
    pos, tpad, nt = cfg["pos"], cfg["tpad"], cfg["nt"]
    h0 = H_LOC * c
    ksl = slice(KSLICE * c, KSLICE * (c + 1))

    xt = _bf(x.reshape(B, D_MODEL).T.reshape(32, 128, B)
             .transpose(1, 0, 2).reshape(128, 32 * B))

    def wrow(W, scale=1.0):
        # W_slice^T [4096 k, 512 j] -> [128, 32*512] with 32 k-chunks packed
        # contiguously along each partition row
        return (W[ksl, :].T * scale).reshape(32, 128, 512).transpose(1, 0, 2) \
            .reshape(128, 32 * 512)

    wqkv = _bf(np.concatenate(
        [wrow(Wq, SCALE), wrow(Wk), wrow(Wv)], axis=1))

    # Wo^T slice [512 k, 4096 j] -> [128 d, (8 n, 4 h, 512 j)]
    wo_t = _bf(Wo[:, ksl].T.reshape(H_LOC, 128, 8, 512)
               .transpose(1, 2, 0, 3).reshape(128, 32 * 512))

    kt = np.empty((128, cfg["sumk"]), dtype=ml_dtypes.bfloat16)
    vg = np.empty((128, cfg["sumv"]), dtype=ml_dtypes.bfloat16)
    for b in range(B):
        nb = tpad[b] // BLOCK_SIZE
        blocks = np.asarray(block_tables[b, :nb])
        kb = key_cache[blocks][:, :, h0: h0 + H_LOC, :].reshape(
            tpad[b], H_LOC, HEAD_DIM)
        vb = value_cache[blocks][:, :, h0: h0 + H_LOC, :].reshape(
            tpad[b], H_LOC, HEAD_DIM)
        kt[:, cfg["kofs"][b]: cfg["kofs"][b] + 4 * tpad[b]] = _bf(
            kb.transpose(2, 1, 0).reshape(HEAD_DIM, H_LOC * tpad[b]))
        # V: [tpad, 512] -> [128 p, nt c, 512 f] row-contiguous
        vg[:, cfg["vofs"][b]: cfg["vofs"][b] + 512 * nt[b]] = _bf(
            vb.reshape(nt[b], 128, KSLICE).transpose(1, 0, 2)
            .reshape(128, nt[b] * KSLICE))
    return {"xt": xt, "wqkv": wqkv, "wo_t": wo_t, "kt": kt, "vg": vg}


def kernel(x, Wq, Wk, Wv, Wo, key_cache, value_cache, block_tables, positions,
           _trace=False):
    x = np.asarray(x, dtype=np.float32)
    Wq = np.asarray(Wq, dtype=np.float32)
    Wk = np.asarray(Wk, dtype=np.float32)
    Wv = np.asarray(Wv, dtype=np.float32)
    Wo = np.asarray(Wo, dtype=np.float32)
    key_cache = np.asarray(key_cache, dtype=np.float32)
    value_cache = np.asarray(value_cache, dtype=np.float32)
    block_tables = np.asarray(block_tables)
    positions = np.asarray(positions)

    cfg = _cfg_from_positions(positions)
    nc = _get_program(cfg)

    in_maps = [
        make_core_inputs(cfg, c, x, Wq, Wk, Wv, Wo, key_cache, value_cache,
                         block_tables)
        for c in range(N_CORES)
    ]
    res = run_bass_kernel_spmd(nc, in_maps, core_ids=list(range(N_CORES)))
    out = np.zeros((B, D_MODEL), dtype=np.float32)
    for r in res.results:
        out += r["out_part"]
    kernel.last_results = res
    return out.reshape(B, 1, D_MODEL).astype(np.float32)
